# revision 1
# baseline (speedup 1.0000x reference)
"""Trainium2 Bass kernel for nn_LinearEmbedded (moe_routing).

Reference computation:
    w = weight1[region_ix]             # (B, C, D) gather per-region weights
    out = einsum('abc,bcd->abd', x, w) + bias1[region_ix][None]

Sharding: B (128 regions) split across 8 NeuronCores, 16 per core; the
per-region weight/bias gather happens host-side.

int8 weight path: each w_b row (fixed c) is quantized host-side to int8
with a per-row scale s_bc = max|w_bc|/127 folded into x
(x'_abc = x_abc * s_bc, fp16).  The device converts int8 -> fp16 on DVE
(exact for |q|<=127); the fp16 matmul x' @ q reproduces x @ w with l2 rel
err ~7.4e-3 (resid_var 5.5e-5).  Per-core HBM ~8.7 MB.

v5 schedule (driven by v1-v4 trace analysis):
  - DMA completion (issue -> semaphore) latency is ~3 us, so the critical
    fill path avoids the DMA->convert->matmul double hop: b0/b1 arrive
    pre-converted fp16 (wf01) and feed the PE directly; DVE conversion
    starts at b2.  4 dummy matmuls bridge preamble-end to b0-data so the
    HAM clock gate (1.2 -> 2.4 GHz after ~3.4 us sustained PE activity)
    flips early and stays warm.
  - each HWDGE dma_start costs ~0.7 us of issuing-engine time: w8 rides
    in 2-b 512 KB pairs, xt in 4-b 512 KB quads (3 slots, first three
    issued immediately), outputs in 2-b pair stores.
  - PSUM evacuated pair-wise on ScalarE (one ACTIVATE over 2 adjacent
    banks of one 6-bank psum allocation); 3 out tiles absorb the ~2.5 us
    store-retire latency; last pair split per-b to shorten the tail.
  - the SP stream ends with the completion proof (every DMA retired,
    every stream drained): ending streams with DMAs in flight lets the
    runtime teardown reset active rings and hard-faults the device
    (NRT_EXEC_UNIT_UNRECOVERABLE, observed on hardware).

Engine roles:
    sync   - wf01 + w8 loads (HWDGE SP ring)
    vector - int8 -> fp16 converts for b2..b15 (one [128,2048] copy per b)
    tensor - warmup dummies, then 4 K=128 matmuls + K=1 bias matmul per b
    scalar - bias + xt loads + paired PSUM evac + out stores (ACT ring)

Per-slot DMA semaphores have at most one outstanding transfer each, so
per-slot counting is exact despite out-of-order queue completion.
"""

import numpy as np

A, B, C, D = 128, 128, 512, 512
NCORES = 8
BL = B // NCORES
KC = C // 128
NP = BL // 2   # 2-b pairs
NQ = BL // 4   # xt quads
R_W8P = 4      # w8 pair slots
R_WF = 8       # fp16 w slots
R_PB = 6       # psum banks for real work (+1 warmup dummy bank)
R_OT = 3       # out tile slots (2-b wide each)
N_WARM = 4     # dummy warmup matmuls

_prog = None


def _build_program():
    global _prog
    if _prog is not None:
        return _prog

    import concourse.bass as bass
    import concourse.mybir as mybir
    from contextlib import ExitStack

    F32 = mybir.dt.float32
    F16 = mybir.dt.float16
    I8 = mybir.dt.int8
    nc = bass.Bass("TRN2", target_bir_lowering=False, debug=False)
    WB = KC * D  # 2048 elements per b per partition

    # b0/b1 weights pre-converted to fp16 host-side (critical fill path)
    wf01 = nc.dram_tensor("wf01", [2, 128, WB], F16, kind="ExternalInput")
    # remaining weights int8, packed in 2-b pairs: rows p=0..6 are b=2p+2,2p+3
    w8 = nc.dram_tensor("w8", [NP - 1, 128, 2 * WB], I8, kind="ExternalInput")
    # xq rows pack 4 b-slices: [b|b+1|b+2|b+3], each a KC*A=512 f16 chunk
    xq = nc.dram_tensor("xq", [NQ, 128, 4 * KC * A], F16, kind="ExternalInput")
    bias = nc.dram_tensor("bias", [1, BL * D + A], F16, kind="ExternalInput")
    # out is [A, BL*D] (partition-major) so 2-b pair stores are a plain
    # 2D->2D copy whose flat element order matches the SBUF tile exactly
    out = nc.dram_tensor("out", [A, BL * D], F16, kind="ExternalOutput")

    ctx = ExitStack()
    with ctx:
        xqs = [
            ctx.enter_context(nc.sbuf_tensor(f"xqs{i}", [128, 4 * KC * A], F16))
            for i in range(3)
        ]
        w8s = [
            ctx.enter_context(nc.sbuf_tensor(f"w8s{i}", [128, 2 * WB], I8))
            for i in range(R_W8P)
        ]
        wfs = [
            ctx.enter_context(nc.sbuf_tensor(f"wfs{i}", [128, WB], F16))
            for i in range(R_WF)
        ]
        ots = [
            ctx.enter_context(nc.sbuf_tensor(f"ots{i}", [128, 2 * D], F16))
            for i in range(R_OT)
        ]
        bias_t = ctx.enter_context(nc.sbuf_tensor("bias_t", [1, BL * D + A], F16))
        psum = ctx.enter_context(nc.psum_tensor("psum", [A, R_PB * D], F32))
        psum_d = ctx.enter_context(nc.psum_tensor("psum_d", [A, D], F32))

        s_wfa = ctx.enter_context(nc.semaphore("s_wfa"))  # b0 fp16 weights
        s_wfb = ctx.enter_context(nc.semaphore("s_wfb"))  # b1 fp16 weights
        s_wp = [ctx.enter_context(nc.semaphore(f"s_wp{i}")) for i in range(R_W8P)]
        s_xa = ctx.enter_context(nc.semaphore("s_xa"))  # b0 xt
        s_xb = ctx.enter_context(nc.semaphore("s_xb"))  # b1-b3 xt
        s_xq1 = ctx.enter_context(nc.semaphore("s_xq1"))
        s_xq2 = ctx.enter_context(nc.semaphore("s_xq2"))
        s_xq3 = ctx.enter_context(nc.semaphore("s_xq3"))
        s_op = [ctx.enter_context(nc.semaphore(f"s_op{i}")) for i in range(R_OT)]
        s_wf = ctx.enter_context(nc.semaphore("s_wf"))  # 2 units per converted b
        s_b = ctx.enter_context(nc.semaphore("s_b"))
        s_pe = ctx.enter_context(nc.semaphore("s_pe"))  # 1 per b
        s_cp = ctx.enter_context(nc.semaphore("s_cp"))  # 1 per evac op
        s_done = ctx.enter_context(nc.semaphore("s_done"))
        all_sems = (
            [s_wfa, s_wfb]
            + s_wp
            + [s_xa, s_xb, s_xq1, s_xq2, s_xq3]
            + s_op
            + [s_wf, s_b, s_pe, s_cp, s_done]
        )

        sync, scalar, tensor, vector = nc.sync, nc.scalar, nc.tensor, nc.vector

        # --- SP engine: wf01 + w8 pair loads, stream ends bare ---
        if True:
            sync.dma_start(wfs[0][:], wf01[0, :, :]).then_inc(s_wfa, 16)
            sync.dma_start(wfs[1][:], wf01[1, :, :]).then_inc(s_wfb, 16)
            for p in range(NP - 1):  # pair p holds b=2p+2, 2p+3
                slot = (p + 1) % R_W8P
                if p >= 4:
                    # slot reused from pair p-4: that pair's 2nd b is
                    # b=2(p-4)+3; converted when s_wf = 2*(2(p-4)+3) - 2
                    sync.wait_ge(s_wf, 4 * p - 12)
                sync.dma_start(w8s[slot][:], w8[p, :, :]).then_inc(s_wp[slot], 16)

            # tail: prove every DMA retired and every stream drained before
            # the program ends -- the runtime teardown resets the DMA rings,
            # and in-flight descriptors at that point hard-fault the device
            # (NRT_EXEC_UNIT_UNRECOVERABLE, observed).
            sync.wait_ge(s_pe, BL)
            sync.wait_ge(s_cp, 9)
            sync.wait_ge(s_wf, 30)
            sync.wait_ge(s_b, 16)
            sync.wait_ge(s_wfa, 16)
            sync.wait_ge(s_wfb, 16)
            sync.wait_ge(s_wp[0], 16)   # pair 3
            sync.wait_ge(s_wp[1], 32)   # pairs 0, 4
            sync.wait_ge(s_wp[2], 32)   # pairs 1, 5
            sync.wait_ge(s_wp[3], 32)   # pairs 2, 6
            sync.wait_ge(s_xa, 16)
            sync.wait_ge(s_xb, 16)
            sync.wait_ge(s_xq1, 16)
            sync.wait_ge(s_xq2, 16)
            sync.wait_ge(s_xq3, 16)
            sync.wait_ge(s_op[0], 48)   # pairs 0, 3, 6
            sync.wait_ge(s_op[1], 48)   # pairs 1, 4 + b14
            sync.wait_ge(s_op[2], 48)   # pairs 2, 5 + b15
            sync.wait_ge(s_done, 3)

        # --- DVE engine: int8 -> fp16 converts for b2..b15 ---
        if True:
            for b in range(2, BL):
                p = b // 2 - 1  # w8 pair row
                half = b % 2
                slot = (p + 1) % R_W8P
                fslot = b % R_WF
                if b >= R_WF:
                    vector.wait_ge(s_pe, b - R_WF + 1)
                vector.wait_ge(s_wp[slot], 16 * (p // R_W8P + 1))
                nc.vector.tensor_copy(
                    wfs[fslot][:], w8s[slot][:, half * WB : (half + 1) * WB]
                ).then_inc(s_wf, 2)
            # trailing scratch convert: gives b15's consumer the same
            # one-convert slack as every other b (see PE lag-by-one wait).
            # w8s[0] (pair b8/b9) is long dead by now; write into its head.
            nc.vector.tensor_copy(
                w8s[0][:, 0:256], w8s[0][:, 256:512]
            ).then_inc(s_wf, 2)
            vector.sem_inc(s_done, 1)

        # --- PE engine: warmup dummies + per-b matmuls ---
        if True:
            ones = bias_t[:, BL * D : BL * D + A]
            for i in range(N_WARM):
                nc.tensor.matmul(
                    psum_d[:], xqs[0][:, 0:A], wfs[2][:, 0:D], start=True, stop=True
                )
            for b in range(BL):
                q, fslot = b // 4, b % R_WF
                if b >= R_PB:
                    tensor.wait_ge(s_cp, (b - R_PB) // 2 + 1)
                if b == 0:
                    tensor.wait_ge(s_xa, 16)
                    tensor.wait_ge(s_wfa, 16)
                elif b == 1:
                    tensor.wait_ge(s_xb, 16)
                    tensor.wait_ge(s_wfb, 16)
                else:
                    if b == 4:
                        tensor.wait_ge(s_xq1, 16)
                    elif b == 8:
                        tensor.wait_ge(s_xq2, 16)
                    elif b == 12:
                        tensor.wait_ge(s_xq3, 16)
                    # lag-by-one: wait until b+1's convert (or the trailing
                    # scratch convert for b15) completed, so b's weights have
                    # been stable in SBUF for a full convert duration -- the
                    # PE read port was observed to race a just-finished DVE
                    # write when it consumes the instant s_wf flips
                    tensor.wait_ge(s_wf, 2 * b)
                for k in range(KC):
                    nc.tensor.matmul(
                        psum[:, (b % R_PB) * D : (b % R_PB) * D + D],
                        xqs[q % 3][
                            :,
                            (b % 4) * KC * A + k * A : (b % 4) * KC * A + (k + 1) * A,
                        ],
                        wfs[fslot][:, k * D : (k + 1) * D],
                        start=(k == 0),
                        stop=False,
                    )
                if b == 0:
                    tensor.wait_ge(s_b, 16)
                nc.tensor.matmul(
                    psum[:, (b % R_PB) * D : (b % R_PB) * D + D],
                    ones,
                    bias_t[:, b * D : (b + 1) * D],
                    start=False,
                    stop=True,
                ).then_inc(s_pe, 1)
            tensor.sem_inc(s_done, 1)

        # --- ACT engine: bias + xt loads + paired PSUM evac + stores ---
        if True:
            scalar.dma_start(bias_t[:], bias[:]).then_inc(s_b, 16)
            scalar.dma_start(xqs[0][:, 0 : KC * A], xq[0, :, 0 : KC * A]).then_inc(
                s_xa, 16
            )
            scalar.dma_start(
                xqs[0][:, KC * A : 4 * KC * A], xq[0, :, KC * A : 4 * KC * A]
            ).then_inc(s_xb, 16)
            scalar.dma_start(xqs[1][:], xq[1, :, :]).then_inc(s_xq1, 16)
            scalar.dma_start(xqs[2][:], xq[2, :, :]).then_inc(s_xq2, 16)
            for p in range(7):
                oslot = p % R_OT
                if p >= R_OT:
                    scalar.wait_ge(s_op[oslot], 16 * ((p - R_OT) // R_OT + 1))
                scalar.wait_ge(s_pe, 2 * p + 2)
                nc.scalar.copy(
                    ots[oslot][:],
                    psum[:, (2 * p % R_PB) * D : (2 * p % R_PB) * D + 2 * D],
                ).then_inc(s_cp, 1)
                # the store's SDMA read of ots does NOT inherit engine-FIFO
                # ordering from the copy (race detector + HW-verified): an
                # explicit wait on the copy's completion sem is required
                scalar.wait_ge(s_cp, p + 1)
                scalar.dma_start(
                    out[:, 2 * p * D : (2 * p + 2) * D], ots[oslot][:]
                ).then_inc(s_op[oslot], 16)
                if p == 1:  # s_pe >= 4 held: quad 0 consumed, slot 0 free
                    scalar.dma_start(xqs[0][:], xq[3, :, :]).then_inc(s_xq3, 16)
            # last pair split per-b: b14 -> ots[1], b15 -> ots[2]
            scalar.wait_ge(s_op[1], 32)
            scalar.wait_ge(s_pe, 15)
            nc.scalar.copy(
                ots[1][:, 0:D], psum[:, (14 % R_PB) * D : (14 % R_PB) * D + D]
            ).then_inc(s_cp, 1)
            scalar.wait_ge(s_cp, 8)
            scalar.dma_start(out[:, 14 * D : 15 * D], ots[1][:, 0:D]).then_inc(
                s_op[1], 16
            )
            scalar.wait_ge(s_op[2], 32)
            scalar.wait_ge(s_pe, 16)
            nc.scalar.copy(
                ots[2][:, 0:D], psum[:, (15 % R_PB) * D : (15 % R_PB) * D + D]
            ).then_inc(s_cp, 1)
            scalar.wait_ge(s_cp, 9)
            scalar.dma_start(out[:, 15 * D : 16 * D], ots[2][:, 0:D]).then_inc(
                s_op[2], 16
            )
            scalar.sem_inc(s_done, 1)

    _prog = nc
    return nc


def _shard_inputs(x, region_ix, weight1, bias1):
    in_maps = []
    for c in range(NCORES):
        bs = slice(c * BL, (c + 1) * BL)
        rloc = region_ix[bs]
        wg = weight1[rloc]  # (BL, C, D) f32
        # per-row int8 quantization; scale folded into x below
        s = np.maximum(np.abs(wg).max(axis=2), 1e-30) / 127.0  # (BL, C)
        q = np.clip(np.rint(wg / s[:, :, None]), -127, 127).astype(np.int8)
        wdev = np.ascontiguousarray(
            q.reshape(BL, KC, 128, D).transpose(0, 2, 1, 3)
        ).reshape(BL, 128, KC * D)
        wf01 = wdev[0:2].astype(np.float16)  # b0/b1 pre-converted
        w8v = np.ascontiguousarray(
            wdev[2:].reshape(NP - 1, 2, 128, KC * D).transpose(0, 2, 1, 3)
        ).reshape(NP - 1, 128, 2 * KC * D)
        xs = (x[:, bs, :] * s[None, :, :]).astype(np.float16)  # (A, BL, C)
        xsv = np.ascontiguousarray(xs.transpose(1, 2, 0))  # (BL, C, A)
        xtv = np.ascontiguousarray(
            xsv.reshape(BL, KC, 128, A).transpose(0, 2, 1, 3)
        ).reshape(BL, 128, KC * A)
        xqv = np.ascontiguousarray(
            xtv.reshape(NQ, 4, 128, KC * A).transpose(0, 2, 1, 3)
        ).reshape(NQ, 128, 4 * KC * A)
        bg = np.concatenate(
            [bias1[rloc].astype(np.float16).reshape(BL * D), np.ones(A, np.float16)]
        ).reshape(1, BL * D + A)
        in_maps.append({"wf01": wf01, "w8": w8v, "xq": xqv, "bias": bg})
    return in_maps


def kernel(x, region_ix, weight1, bias1):
    from concourse.bass_utils import run_bass_kernel_spmd

    x = np.asarray(x, dtype=np.float32)
    region_ix = np.asarray(region_ix).astype(np.int64)
    weight1 = np.asarray(weight1, dtype=np.float32)
    bias1 = np.asarray(bias1, dtype=np.float32)

    nc = _build_program()
    in_maps = _shard_inputs(x, region_ix, weight1, bias1)
    res = run_bass_kernel_spmd(nc, in_maps, core_ids=list(range(NCORES)))

    outv = np.empty((A, B, D), dtype=np.float32)
    for c in range(NCORES):
        outv[:, c * BL : (c + 1) * BL, :] = (
            res.results[c]["out"].reshape(A, BL, D).astype(np.float32)
        )
    return outv



# revision 2
# speedup vs baseline: 1.2601x; 1.2601x over previous
"""Trainium2 Bass kernel for nn_LinearEmbedded (moe_routing).

Reference computation:
    w = weight1[region_ix]             # (B, C, D) gather per-region weights
    out = einsum('abc,bcd->abd', x, w) + bias1[region_ix][None]

Sharding: B (128 regions) split across 8 NeuronCores, 16 per core; the
per-region weight/bias gather happens host-side.

v6 scheme (replaces v5's int8+DVE-convert path):
  - Weights ship as fp8 e3m4 (1 B/elem) with a per-(b,c)-row scale folded
    into x; the PE consumes e3m4 as the MOVING operand directly (validated
    bit-exact on HW vs numpy, probe_fp8.py), so the ~30 us of DVE
    int8->fp16 weight converts that gated v5 disappear entirely.  The
    row scale is picked per-row from 6 candidates to minimize l2 error.
  - x ships as int8 (per-(a,b)-row scale t, also folded out on the host)
    and is cast int8->fp16 on the otherwise-idle DVE (8 pair casts).
  - bias add and the t_ab un-scaling happen host-side after download, so
    the PE runs only the 64 K=128 matmuls (N=512 rows each) -- no bias
    matmuls.  PSUM (|max| ~24e3) evacuates as fp16 (fits, adds <0.1% err).
  - l2 rel err vs the fp32 reference: 1.49e-2 (gate 2e-2), dominated by
    the e3m4 mantissa (4 bits); x-int8 contributes 0.68%.

Schedule (per core, all HBM traffic 7.34 MB ~= 20.4 us at 360 GB/s):
  - SP issues the 12 load DMAs in stream order [x0 w0 x1 w1 w2 x2 w3 w4
    x3 w5 w6 w7] (w pairs 0.52 MB, x quads 0.26 MB); every load has a
    dedicated SBUF region (no reuse, no flow control).  HAM stays warm:
    ~11 dummy matmuls bridge t=0 to the first real weights, and the PE
    consumes b's slower (0.85 us) than the stream delivers them (0.73),
    so it never idles a >3.4 us HAM window once started.
  - DVE pair-casts x with lag-by-one consumption on the PE (wait cast
    b/2+2, trailing scratch cast) -- the PE read port races a
    just-finished DVE write if it consumes the instant the sem flips
    (observed on HW in v5).
  - ACT evacuates PSUM pair-wise (fp32->fp16, [128,1024] per ACTIVATE),
    Pool stores pairs via SWDGE (994 ns gen on Pool, no shared-HWDGE
    contention with the loads).
  - SP ends with the completion proof (every DMA retired, every stream
    drained): teardown resets active DMA rings and in-flight descriptors
    hard-fault the device (NRT_EXEC_UNIT_UNRECOVERABLE, observed).
"""

import numpy as np
import ml_dtypes

A, B, C, D = 128, 128, 512, 512
NCORES = 8
BL = B // NCORES   # 16 b per core
KC = C // 128      # 4 contraction chunks
NP = BL // 2       # 8 w pairs
NQ = BL // 4       # 4 x quads
R_PB = 6           # psum banks for real work (+1 warmup dummy bank)
R_OT = 3           # out tile slots (2-b wide each)
N_WARM = 11        # dummy warmup matmuls (HAM clock ramp)
F8MAX = 15.5       # e3m4 max finite
WB = KC * D        # 2048 weight cols per b
XB = KC * A        # 512 x cols per b

_prog = None


def _build_program():
    global _prog
    if _prog is not None:
        return _prog

    import concourse.bass as bass
    import concourse.mybir as mybir
    from contextlib import ExitStack

    F32 = mybir.dt.float32
    F16 = mybir.dt.float16
    F8 = mybir.dt.float8e3
    I8 = mybir.dt.int8
    nc = bass.Bass("TRN2", target_bir_lowering=False, debug=False)

    w8 = nc.dram_tensor("w8", [NP, 128, 2 * WB], F8, kind="ExternalInput")
    xq = nc.dram_tensor("xq", [NQ, 128, 4 * XB], I8, kind="ExternalInput")
    out = nc.dram_tensor("out", [A, BL * D], F16, kind="ExternalOutput")

    ctx = ExitStack()
    with ctx:
        ws = ctx.enter_context(nc.sbuf_tensor("ws", [128, BL * WB], F8))
        xs8 = ctx.enter_context(nc.sbuf_tensor("xs8", [128, BL * XB], I8))
        xs = ctx.enter_context(nc.sbuf_tensor("xs", [128, BL * XB], F16))
        ots = [
            ctx.enter_context(nc.sbuf_tensor(f"ots{i}", [128, 2 * D], F16))
            for i in range(R_OT)
        ]
        wrm = ctx.enter_context(nc.sbuf_tensor("wrm", [128, 128 + D], F16))
        psum = ctx.enter_context(nc.psum_tensor("psum", [A, R_PB * D], F32))
        psum_d = ctx.enter_context(nc.psum_tensor("psum_d", [A, D], F32))

        s_w = [ctx.enter_context(nc.semaphore(f"s_w{p}")) for p in range(NP)]
        s_x = [ctx.enter_context(nc.semaphore(f"s_x{q}")) for q in range(NQ)]
        s_st = [ctx.enter_context(nc.semaphore(f"s_st{i}")) for i in range(R_OT)]
        s_xc = ctx.enter_context(nc.semaphore("s_xc"))  # +1 per DVE cast
        s_pe = ctx.enter_context(nc.semaphore("s_pe"))  # +1 per finished b
        s_cp = ctx.enter_context(nc.semaphore("s_cp"))  # +1 per PSUM evac

        sync, scalar, tensor, vector, pool = (
            nc.sync, nc.scalar, nc.tensor, nc.vector, nc.gpsimd,
        )

        # --- SP: all 12 loads, no flow control (distinct SBUF regions) ---
        # stream order paces the PE: x quads slotted one pair early
        order = ["x0", "w0", "x1", "w1", "w2", "x2", "w3", "w4", "x3",
                 "w5", "w6", "w7"]
        for item in order:
            i = int(item[1])
            if item[0] == "x":
                sync.dma_start(
                    xs8[:, i * 4 * XB : (i + 1) * 4 * XB], xq[i, :, :]
                ).then_inc(s_x[i], 16)
            else:
                sync.dma_start(
                    ws[:, i * 2 * WB : (i + 1) * 2 * WB], w8[i, :, :]
                ).then_inc(s_w[i], 16)

        # tail: completion proof (see module docstring)
        sync.wait_ge(s_pe, BL)
        sync.wait_ge(s_cp, NP)
        sync.wait_ge(s_xc, NP + 1)
        for p in range(NP):
            sync.wait_ge(s_w[p], 16)
        for q in range(NQ):
            sync.wait_ge(s_x[q], 16)
        sync.wait_ge(s_st[0], 48)  # pairs 0,3,6
        sync.wait_ge(s_st[1], 48)  # pairs 1,4,7
        sync.wait_ge(s_st[2], 32)  # pairs 2,5

        # --- DVE: 8 pair casts int8 -> fp16 + trailing scratch cast ---
        for c in range(NP):
            vector.wait_ge(s_x[c // 2], 16)
            nc.vector.tensor_copy(
                xs[:, c * 2 * XB : (c + 1) * 2 * XB],
                xs8[:, c * 2 * XB : (c + 1) * 2 * XB],
            ).then_inc(s_xc, 1)
        # scratch cast: gives pair 7's consumer the same lag-by-one slack
        # (xs8[:, :64] is dead -- cast 0 already consumed it)
        nc.vector.tensor_copy(xs8[:, 0:64], xs8[:, 64:128]).then_inc(s_xc, 1)

        # --- PE: warmup dummies + 4 K=128 matmuls per b ---
        for _ in range(N_WARM):
            nc.tensor.matmul(
                psum_d[:], wrm[:, 0:128], wrm[:, 128 : 128 + D],
                start=True, stop=True,
            )
        for b in range(BL):
            if b >= R_PB:
                tensor.wait_ge(s_cp, (b - R_PB) // 2 + 1)
            if b % 2 == 0:
                tensor.wait_ge(s_w[b // 2], 16)
                tensor.wait_ge(s_xc, b // 2 + 2)  # lag-by-one on DVE casts
            for k in range(KC):
                mm = nc.tensor.matmul(
                    psum[:, (b % R_PB) * D : (b % R_PB) * D + D],
                    xs[:, b * XB + k * A : b * XB + (k + 1) * A],
                    ws[:, b * WB + k * D : b * WB + (k + 1) * D],
                    start=(k == 0),
                    stop=(k == KC - 1),
                )
            mm.then_inc(s_pe, 1)

        # --- ACT: pair-wise PSUM evac fp32 -> fp16 ---
        for p in range(NP):
            slot = p % R_OT
            if p >= R_OT:
                scalar.wait_ge(s_st[slot], 16 * (p // R_OT))
            scalar.wait_ge(s_pe, 2 * p + 2)
            nc.scalar.copy(
                ots[slot][:],
                psum[:, (2 * p % R_PB) * D : (2 * p % R_PB) * D + 2 * D],
            ).then_inc(s_cp, 1)

        # --- Pool: pair stores via SWDGE ---
        for p in range(NP):
            slot = p % R_OT
            pool.wait_ge(s_cp, p + 1)
            pool.dma_start(
                out[:, 2 * p * D : (2 * p + 2) * D], ots[slot][:]
            ).then_inc(s_st[slot], 16)

    _prog = nc
    return nc


_RATIOS = np.array([1.0, 0.97, 0.94, 0.91, 0.88, 0.85], dtype=np.float32)


def _quant_w_e3m4(wg):
    """Per-(b,c)-row e3m4 quantization with l2-optimal scale from 6
    candidates.  Returns (qw float8_e3m4 (BL,C,D), s (BL,C) fp32)."""
    f8 = ml_dtypes.float8_e3m4
    wmax = np.maximum(np.abs(wg).max(axis=2), 1e-30)  # (BL,C)
    best_err = None
    best_s = None
    best_q = None
    for r in _RATIOS:
        s = (wmax / (F8MAX * r)).astype(np.float32)
        q = np.clip(wg / s[:, :, None], -F8MAX, F8MAX).astype(f8)
        e = ((q.astype(np.float32) * s[:, :, None] - wg) ** 2).sum(axis=2)
        if best_err is None:
            best_err, best_s, best_q = e, s, q
        else:
            m = e < best_err
            best_err = np.where(m, e, best_err)
            best_s = np.where(m, s, best_s)
            best_q[m] = q[m]
    return best_q, best_s


def _shard_inputs(x, region_ix, weight1, bias1):
    in_maps = []
    post = []  # (t, bias) per core for host-side un-scaling
    for c in range(NCORES):
        bs = slice(c * BL, (c + 1) * BL)
        rloc = region_ix[bs]
        wg = weight1[rloc]                        # (BL, C, D) f32
        qw, s = _quant_w_e3m4(wg)
        # device layout: per b [128 part (c within chunk), KC*D], pairs
        wdev = np.ascontiguousarray(
            qw.reshape(BL, KC, 128, D).transpose(0, 2, 1, 3)
        ).reshape(BL, 128, WB)
        w8v = np.ascontiguousarray(
            wdev.reshape(NP, 2, 128, WB).transpose(0, 2, 1, 3)
        ).reshape(NP, 128, 2 * WB)
        # x: fold s, int8 per-(a,b)-row
        xp = x[:, bs, :] * s[None, :, :]          # (A, BL, C)
        t = np.maximum(np.abs(xp).max(axis=2), 1e-30) / 127.0   # (A, BL)
        qx = np.clip(np.rint(xp / t[:, :, None]), -127, 127).astype(np.int8)
        # device layout: per b [128 part (c within chunk), KC*A], quads
        xt = np.ascontiguousarray(
            qx.transpose(1, 2, 0).reshape(BL, KC, 128, A).transpose(0, 2, 1, 3)
        ).reshape(BL, 128, XB)
        xqv = np.ascontiguousarray(
            xt.reshape(NQ, 4, 128, XB).transpose(0, 2, 1, 3)
        ).reshape(NQ, 128, 4 * XB)
        in_maps.append({"w8": w8v, "xq": xqv})
        post.append((t, bias1[rloc]))
    return in_maps, post


def kernel(x, region_ix, weight1, bias1):
    from concourse.bass_utils import run_bass_kernel_spmd

    x = np.asarray(x, dtype=np.float32)
    region_ix = np.asarray(region_ix).astype(np.int64)
    weight1 = np.asarray(weight1, dtype=np.float32)
    bias1 = np.asarray(bias1, dtype=np.float32)

    nc = _build_program()
    in_maps, post = _shard_inputs(x, region_ix, weight1, bias1)
    res = run_bass_kernel_spmd(nc, in_maps, core_ids=list(range(NCORES)))

    outv = np.empty((A, B, D), dtype=np.float32)
    for c in range(NCORES):
        t, bg = post[c]
        acc = res.results[c]["out"].reshape(A, BL, D).astype(np.float32)
        outv[:, c * BL : (c + 1) * BL, :] = acc * t[:, :, None] + bg[None]
    return outv


# revision 3
# speedup vs baseline: 1.3742x; 1.0906x over previous
"""Trainium2 Bass kernel for nn_LinearEmbedded (moe_routing).

Reference computation:
    w = weight1[region_ix]             # (B, C, D) gather per-region weights
    out = einsum('abc,bcd->abd', x, w) + bias1[region_ix][None]

Sharding: B (128 regions) split across 8 NeuronCores, 16 per core; the
per-region weight/bias gather happens host-side.

v7 scheme (v6 + tail/line tuning; v6 replaced v5's int8+DVE-convert path):
  - Weights ship as fp8 e3m4 (1 B/elem) with a per-(b,c)-row scale folded
    into x; the PE consumes e3m4 as the MOVING operand directly (validated
    bit-exact on HW vs numpy, probe_fp8.py), so v5's ~30 us of DVE
    int8->fp16 weight converts disappear entirely.  The row scale is
    picked per-row from 6 candidates to minimize l2 error.
  - x ships as int8 (per-(a,b)-row scale t, folded out on the host) and
    is pair-cast int8->fp16 on the otherwise-idle DVE.
  - bias add and the t_ab un-scaling happen host-side after download, so
    the PE runs only the 64 K=128 matmuls (N=512 rows each).  PSUM
    (|max| ~24e3) evacuates as fp16.
  - l2 rel err vs the fp32 reference: 1.49e-2 (gate 2e-2).

Schedule (per core; HBM traffic 7.34 MB ~= 24 us at the measured
306 B/ns pool rate -- the kernel is DMA-bandwidth-bound, v6 trace):
  - SP issues 12 loads in stream order [x0 w01 w23 x1 w45 w67 x2367
    w89 wAB wCD w14 w15]; every load has a dedicated SBUF region (no
    reuse, no flow control).  The last two w chunks are single-b so the
    final matmuls start as early as possible.
  - PE: 13 dummy matmuls bridge the preamble to the first real weights
    (HAM clock-gate warms after ~3.4 us of sustained activity and the
    stream then paces the PE, so it stays at 2.4 GHz -- v6 trace shows
    215 ns/matmul steady-state).
  - DVE pair-casts x with lag-by-one consumption on the PE (wait cast
    b/2+2 + trailing scratch cast) -- the PE read port races a
    just-finished DVE write if it consumes the instant the sem flips
    (observed on HW in v5).
  - ACT evacuates PSUM pair-wise for b0..b11 ([128,1024] ACTIVATEs) and
    per-b for b12..b15 to shorten the tail chain; every output stages in
    a dedicated SBUF region (no slot recycling, no store->evac waits).
  - Pool stores via SWDGE (no shared-HWDGE contention with loads):
    three 4-b stores (4 KB lines) + four 1-b stores at the tail.
  - SP ends with the completion proof (every DMA retired, every stream
    drained): teardown resets active DMA rings and in-flight descriptors
    hard-fault the device (NRT_EXEC_UNIT_UNRECOVERABLE, observed).
  - The remaining ~7 us after the last store sem is the framework
    postamble (exit barrier + clearing all 256 HW semaphores, ~51 per
    engine serially); it is re-executability teardown and not avoidable
    from kernel code.
"""

import numpy as np
import ml_dtypes

A, B, C, D = 128, 128, 512, 512
NCORES = 8
BL = B // NCORES   # 16 b per core
KC = C // 128      # 4 contraction chunks
R_PB = 6           # psum banks for real work (+1 warmup dummy bank)
N_WARM = 13        # dummy warmup matmuls (HAM clock ramp + preamble bridge)
F8MAX = 15.5       # e3m4 max finite
WB = KC * D        # 2048 weight cols per b
XB = KC * A        # 512 x cols per b

# w load chunks: (first b, n bs); last two single so the tail starts early
W_CHUNKS = [(0, 2), (2, 2), (4, 2), (6, 2), (8, 2), (10, 2), (12, 2),
            (14, 1), (15, 1)]
# x load chunks (in units of b)
X_CHUNKS = [(0, 4), (4, 4), (8, 8)]
# out stores: (first b, n bs); singles at the tail
O_CHUNKS = [(0, 4), (4, 4), (8, 4), (12, 1), (13, 1), (14, 1), (15, 1)]
# SP issue order: x chunks slotted so they arrive just ahead of need
LOAD_ORDER = ["x0", "w0", "w1", "x1", "w2", "w3", "x2", "w4", "w5", "w6",
              "w7", "w8"]

_prog = None


def _build_program():
    global _prog
    if _prog is not None:
        return _prog

    import concourse.bass as bass
    import concourse.mybir as mybir
    from contextlib import ExitStack

    F32 = mybir.dt.float32
    F16 = mybir.dt.float16
    F8 = mybir.dt.float8e3
    I8 = mybir.dt.int8
    nc = bass.Bass("TRN2", target_bir_lowering=False, debug=False)

    w8 = nc.dram_tensor("w8", [128, BL * WB], F8, kind="ExternalInput")
    xq = nc.dram_tensor("xq", [128, BL * XB], I8, kind="ExternalInput")
    out = nc.dram_tensor("out", [A, BL * D], F16, kind="ExternalOutput")

    ctx = ExitStack()
    with ctx:
        ws = ctx.enter_context(nc.sbuf_tensor("ws", [128, BL * WB], F8))
        xs8 = ctx.enter_context(nc.sbuf_tensor("xs8", [128, BL * XB], I8))
        xs = ctx.enter_context(nc.sbuf_tensor("xs", [128, BL * XB], F16))
        ots = ctx.enter_context(nc.sbuf_tensor("ots", [128, BL * D], F16))
        wrm = ctx.enter_context(nc.sbuf_tensor("wrm", [128, 128 + D], F16))
        psum = ctx.enter_context(nc.psum_tensor("psum", [A, R_PB * D], F32))
        psum_d = ctx.enter_context(nc.psum_tensor("psum_d", [A, D], F32))

        s_w = [ctx.enter_context(nc.semaphore(f"s_w{p}"))
               for p in range(len(W_CHUNKS))]
        s_x = [ctx.enter_context(nc.semaphore(f"s_x{q}"))
               for q in range(len(X_CHUNKS))]
        s_xc = ctx.enter_context(nc.semaphore("s_xc"))  # +1 per DVE cast
        s_pe = ctx.enter_context(nc.semaphore("s_pe"))  # +1 per finished b
        s_cp = ctx.enter_context(nc.semaphore("s_cp"))  # +1 per PSUM evac
        s_st = ctx.enter_context(nc.semaphore("s_st"))  # +16 per store

        sync, scalar, tensor, vector, pool = (
            nc.sync, nc.scalar, nc.tensor, nc.vector, nc.gpsimd,
        )

        # which w chunk / evac feeds each b
        w_of_b = {}
        for ci, (b0, nb) in enumerate(W_CHUNKS):
            for b in range(b0, b0 + nb):
                w_of_b[b] = ci
        # s_cp threshold at which b's PSUM contents have been evacuated
        evac_after_b = {}
        cp = 0
        for p in range(6):            # pair evacs b0..b11
            cp += 1
            evac_after_b[2 * p] = cp
            evac_after_b[2 * p + 1] = cp
        for b in range(12, 16):       # single evacs b12..b15
            cp += 1
            evac_after_b[b] = cp
        n_evacs = cp

        # --- SP: all loads, no flow control (distinct SBUF regions) ---
        for item in LOAD_ORDER:
            i = int(item[1:])
            if item[0] == "x":
                b0, nb = X_CHUNKS[i]
                sync.dma_start(
                    xs8[:, b0 * XB : (b0 + nb) * XB],
                    xq[:, b0 * XB : (b0 + nb) * XB],
                ).then_inc(s_x[i], 16)
            else:
                b0, nb = W_CHUNKS[i]
                sync.dma_start(
                    ws[:, b0 * WB : (b0 + nb) * WB],
                    w8[:, b0 * WB : (b0 + nb) * WB],
                ).then_inc(s_w[i], 16)

        # tail: completion proof (see module docstring)
        sync.wait_ge(s_pe, BL)
        sync.wait_ge(s_cp, n_evacs)
        sync.wait_ge(s_xc, 9)
        for p in range(len(W_CHUNKS)):
            sync.wait_ge(s_w[p], 16)
        for q in range(len(X_CHUNKS)):
            sync.wait_ge(s_x[q], 16)
        sync.wait_ge(s_st, 16 * len(O_CHUNKS))

        # --- DVE: 8 pair casts int8 -> fp16 + trailing scratch cast ---
        xc_of_pair = []
        for c in range(8):
            xcl = [i for i, (b0, nb) in enumerate(X_CHUNKS)
                   if b0 <= 2 * c < b0 + nb][0]
            xc_of_pair.append(xcl)
            vector.wait_ge(s_x[xcl], 16)
            nc.vector.tensor_copy(
                xs[:, c * 2 * XB : (c + 1) * 2 * XB],
                xs8[:, c * 2 * XB : (c + 1) * 2 * XB],
            ).then_inc(s_xc, 1)
        # scratch cast: gives pair 7's consumer the same lag-by-one slack
        # (xs8[:, :64] is dead -- cast 0 already consumed it)
        nc.vector.tensor_copy(xs8[:, 0:64], xs8[:, 64:128]).then_inc(s_xc, 1)

        # --- PE: warmup dummies + 4 K=128 matmuls per b ---
        for _ in range(N_WARM):
            nc.tensor.matmul(
                psum_d[:], wrm[:, 0:128], wrm[:, 128 : 128 + D],
                start=True, stop=True,
            )
        waited_w = set()
        for b in range(BL):
            if b >= R_PB:
                tensor.wait_ge(s_cp, evac_after_b[b - R_PB])
            wc = w_of_b[b]
            if wc not in waited_w:
                waited_w.add(wc)
                tensor.wait_ge(s_w[wc], 16)
            if b % 2 == 0:
                tensor.wait_ge(s_xc, b // 2 + 2)  # lag-by-one on DVE casts
            for k in range(KC):
                mm = nc.tensor.matmul(
                    psum[:, (b % R_PB) * D : (b % R_PB) * D + D],
                    xs[:, b * XB + k * A : b * XB + (k + 1) * A],
                    ws[:, b * WB + k * D : b * WB + (k + 1) * D],
                    start=(k == 0),
                    stop=(k == KC - 1),
                )
            mm.then_inc(s_pe, 1)

        # --- ACT: PSUM evac fp32 -> fp16 (pairs, then singles at tail) ---
        for p in range(6):
            scalar.wait_ge(s_pe, 2 * p + 2)
            nc.scalar.copy(
                ots[:, 2 * p * D : (2 * p + 2) * D],
                psum[:, (2 * p % R_PB) * D : (2 * p % R_PB) * D + 2 * D],
            ).then_inc(s_cp, 1)
        for b in range(12, 16):
            scalar.wait_ge(s_pe, b + 1)
            nc.scalar.copy(
                ots[:, b * D : (b + 1) * D],
                psum[:, (b % R_PB) * D : (b % R_PB) * D + D],
            ).then_inc(s_cp, 1)

        # --- Pool: stores via SWDGE ---
        for b0, nb in O_CHUNKS:
            pool.wait_ge(s_cp, evac_after_b[b0 + nb - 1])
            pool.dma_start(
                out[:, b0 * D : (b0 + nb) * D], ots[:, b0 * D : (b0 + nb) * D]
            ).then_inc(s_st, 16)

    _prog = nc
    return nc


_RATIOS = np.array([1.0, 0.97, 0.94, 0.91, 0.88, 0.85], dtype=np.float32)


def _quant_w_e3m4(wg):
    """Per-(b,c)-row e3m4 quantization with l2-optimal scale from 6
    candidates.  Returns (qw float8_e3m4 (BL,C,D), s (BL,C) fp32)."""
    f8 = ml_dtypes.float8_e3m4
    wmax = np.maximum(np.abs(wg).max(axis=2), 1e-30)  # (BL,C)
    best_err = None
    best_s = None
    best_q = None
    for r in _RATIOS:
        s = (wmax / (F8MAX * r)).astype(np.float32)
        q = np.clip(wg / s[:, :, None], -F8MAX, F8MAX).astype(f8)
        e = ((q.astype(np.float32) * s[:, :, None] - wg) ** 2).sum(axis=2)
        if best_err is None:
            best_err, best_s, best_q = e, s, q
        else:
            m = e < best_err
            best_err = np.where(m, e, best_err)
            best_s = np.where(m, s, best_s)
            best_q[m] = q[m]
    return best_q, best_s


def _shard_inputs(x, region_ix, weight1, bias1):
    in_maps = []
    post = []  # (t, bias) per core for host-side un-scaling
    for c in range(NCORES):
        bs = slice(c * BL, (c + 1) * BL)
        rloc = region_ix[bs]
        wg = weight1[rloc]                        # (BL, C, D) f32
        qw, s = _quant_w_e3m4(wg)
        # device layout: per b [128 part (c within chunk), KC*D], b-major
        wdev = np.ascontiguousarray(
            qw.reshape(BL, KC, 128, D).transpose(2, 0, 1, 3)
        ).reshape(128, BL * WB)
        # x: fold s, int8 per-(a,b)-row
        xp = x[:, bs, :] * s[None, :, :]          # (A, BL, C)
        t = np.maximum(np.abs(xp).max(axis=2), 1e-30) / 127.0   # (A, BL)
        qx = np.clip(np.rint(xp / t[:, :, None]), -127, 127).astype(np.int8)
        # device layout: per b [128 part (c within chunk), KC*A], b-major
        xt = np.ascontiguousarray(
            qx.transpose(1, 2, 0).reshape(BL, KC, 128, A).transpose(2, 0, 1, 3)
        ).reshape(128, BL * XB)
        in_maps.append({"w8": wdev, "xq": xt})
        post.append((t, bias1[rloc]))
    return in_maps, post


def kernel(x, region_ix, weight1, bias1):
    from concourse.bass_utils import run_bass_kernel_spmd

    x = np.asarray(x, dtype=np.float32)
    region_ix = np.asarray(region_ix).astype(np.int64)
    weight1 = np.asarray(weight1, dtype=np.float32)
    bias1 = np.asarray(bias1, dtype=np.float32)

    nc = _build_program()
    in_maps, post = _shard_inputs(x, region_ix, weight1, bias1)
    res = run_bass_kernel_spmd(nc, in_maps, core_ids=list(range(NCORES)))

    outv = np.empty((A, B, D), dtype=np.float32)
    for c in range(NCORES):
        t, bg = post[c]
        acc = res.results[c]["out"].reshape(A, BL, D).astype(np.float32)
        outv[:, c * BL : (c + 1) * BL, :] = acc * t[:, :, None] + bg[None]
    return outv
